# revision 5
# baseline (speedup 1.0000x reference)
"""Trainium2 Bass kernel for the CoAttention scoring layer.

reference:
    keys    = receiver @ w_k                      # [B, R, D]
    queries = attendant @ w_q                     # [B, A, D]
    e_act   = queries[:, None, :, :] + keys[:, :, None, :] + bias  # [B, R, A, D]
    out     = tanh(e_act) @ a                     # [B, R, A]

Sharding: data-parallel over B across 8 NeuronCores (8 batches per core),
params replicated.

Per-core layout (D=128 lives in the partition dim everywhere):
    kT  (D, R) = w_k^T @ receiver[b]^T   (PE, after PE-transpose of receiver)
    qbT (D, A) = w_q^T @ attendant[b]^T + bias
    per a-index: e[:, a-block] = kT + qbT[:, a]          (DVE tensor_scalar 2x)
    per chunk:   th = tanh(e)                            (ACT, large free dim)
    per a-index: scores[:, a] = th_block^T @ a_vec       (PE matvec, M=R)
    scores (R, A) accumulate in one PSUM region per batch -> DVE copy -> DMA.
"""

import sys

if "/opt/trn_rl_repo" not in sys.path:
    sys.path.insert(0, "/opt/trn_rl_repo")

from contextlib import ExitStack

import numpy as np

import concourse.bacc as bacc
import concourse.bass as bass
import concourse.tile as tile
from concourse import masks, mybir
from concourse.bass_utils import run_bass_kernel_spmd

B, R, A, F = 64, 128, 128, 256
D = F // 2
NCORES = 8
BC = B // NCORES  # batches per core
CA = 32           # a-indices per tanh chunk (ACT free dim = CA * R)
F32 = mybir.dt.float32

_CACHE = {}


def build_bass():
    nc = bacc.Bacc("TRN2", target_bir_lowering=False, debug=False)

    recv = nc.declare_dram_parameter("receiver", [BC, R, F], F32, isOutput=False)
    attn = nc.declare_dram_parameter("attendant", [BC, A, F], F32, isOutput=False)
    wq_d = nc.declare_dram_parameter("w_q", [F, D], F32, isOutput=False)
    wk_d = nc.declare_dram_parameter("w_k", [F, D], F32, isOutput=False)
    bias_d = nc.declare_dram_parameter("bias", [D, 1], F32, isOutput=False)
    a_d = nc.declare_dram_parameter("a", [D, 1], F32, isOutput=False)
    out = nc.declare_dram_parameter("out", [BC, R, A], F32, isOutput=True)

    TANH = mybir.ActivationFunctionType.Tanh

    with tile.TileContext(nc) as tc, ExitStack() as ctx:
        const = ctx.enter_context(tc.tile_pool(name="const", bufs=1))
        natp = ctx.enter_context(tc.tile_pool(name="nat", bufs=2))
        tpp = ctx.enter_context(tc.tile_pool(name="tp", bufs=2, space="PSUM"))
        tsb = ctx.enter_context(tc.tile_pool(name="tsb", bufs=2))
        kqp = ctx.enter_context(tc.tile_pool(name="kqp", bufs=1, space="PSUM"))
        kqs = ctx.enter_context(tc.tile_pool(name="kqs", bufs=2))
        ep = ctx.enter_context(tc.tile_pool(name="ep", bufs=2))
        thp = ctx.enter_context(tc.tile_pool(name="thp", bufs=2))
        scp = ctx.enter_context(tc.tile_pool(name="scp", bufs=2, space="PSUM"))
        scs = ctx.enter_context(tc.tile_pool(name="scs", bufs=2))

        # constants
        identity = const.tile([128, 128], F32, tag="identity")
        masks.make_identity(nc, identity[:])
        wk_sb = const.tile([128, F], F32, tag="wk")   # [f_local, ft*D + d]
        wq_sb = const.tile([128, F], F32, tag="wq")
        for ft in range(2):
            nc.sync.dma_start(wk_sb[:, ft * D:(ft + 1) * D], wk_d[ft * 128:(ft + 1) * 128])
            nc.sync.dma_start(wq_sb[:, ft * D:(ft + 1) * D], wq_d[ft * 128:(ft + 1) * 128])
        bias_col = const.tile([D, 1], F32, tag="bias")
        nc.sync.dma_start(bias_col[:], bias_d[:])
        a_col = const.tile([D, 1], F32, tag="avec")
        nc.sync.dma_start(a_col[:], a_d[:])

        for b in range(BC):
            r_nat = natp.tile([R, F], F32, tag="r_nat")
            nc.sync.dma_start(r_nat[:], recv[b])
            at_nat = natp.tile([A, F], F32, tag="at_nat")
            nc.sync.dma_start(at_nat[:], attn[b])

            # PE transposes: [R, F]-natural -> two (f, r) tiles
            rT = tsb.tile([128, F], F32, tag="rT")
            atT = tsb.tile([128, F], F32, tag="atT")
            for ft in range(2):
                tp0 = tpp.tile([128, 128], F32, tag="tp")
                nc.tensor.transpose(tp0[:], r_nat[:, ft * 128:(ft + 1) * 128], identity[:])
                nc.vector.tensor_copy(rT[:, ft * 128:(ft + 1) * 128], tp0[:])
                tp1 = tpp.tile([128, 128], F32, tag="tp")
                nc.tensor.transpose(tp1[:], at_nat[:, ft * 128:(ft + 1) * 128], identity[:])
                nc.vector.tensor_copy(atT[:, ft * 128:(ft + 1) * 128], tp1[:])

            # kT = w_k^T @ receiver^T   (contract F over two 128-tiles)
            kT_ps = kqp.tile([D, R], F32, tag="kT_ps")
            nc.tensor.matmul(kT_ps[:], wk_sb[:, 0:D], rT[:, 0:128], start=True, stop=False)
            nc.tensor.matmul(kT_ps[:], wk_sb[:, D:2 * D], rT[:, 128:256], start=False, stop=True)
            qT_ps = kqp.tile([D, A], F32, tag="qT_ps")
            nc.tensor.matmul(qT_ps[:], wq_sb[:, 0:D], atT[:, 0:128], start=True, stop=False)
            nc.tensor.matmul(qT_ps[:], wq_sb[:, D:2 * D], atT[:, 128:256], start=False, stop=True)

            kT_sb = kqs.tile([D, R], F32, tag="kT_sb")
            nc.vector.tensor_copy(kT_sb[:], kT_ps[:])
            qbT_sb = kqs.tile([D, A], F32, tag="qbT_sb")
            nc.vector.tensor_scalar_add(qbT_sb[:], qT_ps[:], bias_col[:, 0:1])

            sc_ps = scp.tile([R, A], F32, tag="sc_ps")
            for ac in range(A // CA):
                e = ep.tile([D, CA * R], F32, tag="e")
                th = thp.tile([D, CA * R], F32, tag="th")
                for j in range(CA):
                    aidx = ac * CA + j
                    nc.vector.tensor_scalar_add(
                        e[:, j * R:(j + 1) * R], kT_sb[:], qbT_sb[:, aidx:aidx + 1]
                    )
                nc.scalar.activation(th[:], e[:], TANH)
                for j in range(CA):
                    aidx = ac * CA + j
                    nc.tensor.matmul(
                        sc_ps[:, aidx:aidx + 1],
                        th[:, j * R:(j + 1) * R],
                        a_col[:],
                        start=True,
                        stop=True,
                    )
            sc_sb = scs.tile([R, A], F32, tag="sc_sb")
            nc.vector.tensor_copy(sc_sb[:], sc_ps[:])
            nc.sync.dma_start(out[b], sc_sb[:])

    nc.finalize()
    return nc


def _get_nc():
    if "nc" not in _CACHE:
        _CACHE["nc"] = build_bass()
    return _CACHE["nc"]


def make_in_maps(inputs):
    receiver = np.ascontiguousarray(inputs["receiver"], dtype=np.float32)
    attendant = np.ascontiguousarray(inputs["attendant"], dtype=np.float32)
    w_q = np.ascontiguousarray(inputs["w_q"], dtype=np.float32)
    w_k = np.ascontiguousarray(inputs["w_k"], dtype=np.float32)
    bias = np.ascontiguousarray(inputs["bias"], dtype=np.float32).reshape(D, 1)
    a = np.ascontiguousarray(inputs["a"], dtype=np.float32).reshape(D, 1)
    in_maps = []
    for c in range(NCORES):
        in_maps.append(
            {
                "receiver": np.ascontiguousarray(receiver[c * BC:(c + 1) * BC]),
                "attendant": np.ascontiguousarray(attendant[c * BC:(c + 1) * BC]),
                "w_q": w_q,
                "w_k": w_k,
                "bias": bias,
                "a": a,
            }
        )
    return in_maps


def run(inputs, **kwargs):
    nc = _get_nc()
    in_maps = make_in_maps(inputs)
    res = run_bass_kernel_spmd(nc, in_maps, list(range(NCORES)), **kwargs)
    out = np.concatenate([res.results[c]["out"] for c in range(NCORES)], axis=0)
    return out, res


def kernel(**inputs) -> np.ndarray:
    out, _ = run(inputs)
    return out


# revision 8
# speedup vs baseline: 2.9450x; 2.9450x over previous
"""Trainium2 Bass kernel for the CoAttention scoring layer.

reference:
    keys    = receiver @ w_k                      # [B, R, D]
    queries = attendant @ w_q                     # [B, A, D]
    e_act   = queries[:, None, :, :] + keys[:, :, None, :] + bias  # [B, R, A, D]
    out     = tanh(e_act) @ a                     # [B, R, A]

Sharding: data-parallel over B across 8 NeuronCores (8 batches per core),
params replicated.

Per-core layout (D=128 lives in the partition dim everywhere):
    kT  (D, R) = w_k^T @ receiver[b]^T   (PE, after PE-transpose of receiver)
    qbT (D, A) = w_q^T @ attendant[b]^T + bias
    per a-index: e[:, a-block] = kT + qbT[:, a]          (DVE tensor_scalar 2x)
    per chunk:   th = tanh(e)                            (ACT, large free dim)
    per a-index: scores[:, a] = th_block^T @ a_vec       (PE matvec, M=R)
    scores (R, A) accumulate in one PSUM region per batch -> DVE copy -> DMA.
"""

import sys

if "/opt/trn_rl_repo" not in sys.path:
    sys.path.insert(0, "/opt/trn_rl_repo")

from contextlib import ExitStack

import numpy as np

import concourse.bacc as bacc
import concourse.bass as bass
import concourse.tile as tile
from concourse import masks, mybir
from concourse.bass_utils import run_bass_kernel_spmd

B, R, A, F = 64, 128, 128, 256
D = F // 2
NCORES = 8
BC = B // NCORES  # batches per core
CA = 16           # a-indices per chunk (DVE/ACT free dim = CA * R)
F32 = mybir.dt.float32
F16 = mybir.dt.float16

_CACHE = {}


def build_bass():
    nc = bacc.Bacc("TRN2", target_bir_lowering=False, debug=False)

    recv = nc.declare_dram_parameter("receiver", [BC, R, F], F32, isOutput=False)
    attn = nc.declare_dram_parameter("attendant", [BC, A, F], F32, isOutput=False)
    wq_d = nc.declare_dram_parameter("w_q", [F, D], F32, isOutput=False)
    wk_d = nc.declare_dram_parameter("w_k", [F, D], F32, isOutput=False)
    bias_d = nc.declare_dram_parameter("bias", [D, 1], F32, isOutput=False)
    a_d = nc.declare_dram_parameter("a", [D, 1], F32, isOutput=False)
    out = nc.declare_dram_parameter("out", [BC, R, A], F32, isOutput=True)

    TANH = mybir.ActivationFunctionType.Tanh

    with tile.TileContext(nc) as tc, ExitStack() as ctx:
        const = ctx.enter_context(tc.tile_pool(name="const", bufs=1))
        natp = ctx.enter_context(tc.tile_pool(name="nat", bufs=2))
        tpp = ctx.enter_context(tc.tile_pool(name="tp", bufs=2, space="PSUM"))
        tsb = ctx.enter_context(tc.tile_pool(name="tsb", bufs=2))
        kqp = ctx.enter_context(tc.tile_pool(name="kqp", bufs=1, space="PSUM"))
        kqs = ctx.enter_context(tc.tile_pool(name="kqs", bufs=2))
        ep = ctx.enter_context(tc.tile_pool(name="ep", bufs=2))
        thp = ctx.enter_context(tc.tile_pool(name="thp", bufs=2))
        scp = ctx.enter_context(tc.tile_pool(name="scp", bufs=2, space="PSUM"))
        scs = ctx.enter_context(tc.tile_pool(name="scs", bufs=2))

        # constants
        identity = const.tile([128, 128], F32, tag="identity")
        masks.make_identity(nc, identity[:])
        wk_sb = const.tile([128, F], F32, tag="wk")   # [f_local, ft*D + d]
        wq_sb = const.tile([128, F], F32, tag="wq")
        for ft in range(2):
            nc.sync.dma_start(wk_sb[:, ft * D:(ft + 1) * D], wk_d[ft * 128:(ft + 1) * 128])
            nc.sync.dma_start(wq_sb[:, ft * D:(ft + 1) * D], wq_d[ft * 128:(ft + 1) * 128])
        bias_col = const.tile([D, 1], F32, tag="bias")
        nc.sync.dma_start(bias_col[:], bias_d[:])
        a_col = const.tile([D, 1], F32, tag="avec")
        nc.sync.dma_start(a_col[:], a_d[:])
        a_f16 = const.tile([D, 1], F16, tag="avec16")
        nc.vector.tensor_copy(a_f16[:], a_col[:])

        for b in range(BC):
            r_nat = natp.tile([R, F], F32, tag="r_nat")
            nc.sync.dma_start(r_nat[:], recv[b])
            at_nat = natp.tile([A, F], F32, tag="at_nat")
            nc.sync.dma_start(at_nat[:], attn[b])

            # PE transposes: [R, F]-natural -> two (f, r) tiles
            rT = tsb.tile([128, F], F32, tag="rT")
            atT = tsb.tile([128, F], F32, tag="atT")
            for ft in range(2):
                tp0 = tpp.tile([128, 128], F32, tag="tp")
                nc.tensor.transpose(tp0[:], r_nat[:, ft * 128:(ft + 1) * 128], identity[:])
                nc.vector.tensor_copy(rT[:, ft * 128:(ft + 1) * 128], tp0[:])
                tp1 = tpp.tile([128, 128], F32, tag="tp")
                nc.tensor.transpose(tp1[:], at_nat[:, ft * 128:(ft + 1) * 128], identity[:])
                nc.vector.tensor_copy(atT[:, ft * 128:(ft + 1) * 128], tp1[:])

            # kT = w_k^T @ receiver^T   (contract F over two 128-tiles)
            kT_ps = kqp.tile([D, R], F32, tag="kT_ps")
            nc.tensor.matmul(kT_ps[:], wk_sb[:, 0:D], rT[:, 0:128], start=True, stop=False)
            nc.tensor.matmul(kT_ps[:], wk_sb[:, D:2 * D], rT[:, 128:256], start=False, stop=True)
            qT_ps = kqp.tile([D, A], F32, tag="qT_ps")
            nc.tensor.matmul(qT_ps[:], wq_sb[:, 0:D], atT[:, 0:128], start=True, stop=False)
            nc.tensor.matmul(qT_ps[:], wq_sb[:, D:2 * D], atT[:, 128:256], start=False, stop=True)

            kT_sb = kqs.tile([D, R], F32, tag="kT_sb")
            nc.vector.tensor_copy(kT_sb[:], kT_ps[:])
            qbT_sb = kqs.tile([D, A], F32, tag="qbT_sb")
            nc.vector.tensor_scalar_add(qbT_sb[:], qT_ps[:], bias_col[:, 0:1])

            sc_ps = scp.tile([R, A], F32, tag="sc_ps")
            for ac in range(A // CA):
                a0 = ac * CA
                e = ep.tile([D, CA, R], F32, tag="e")
                th = thp.tile([D, CA, R], F16, tag="th")
                # e[d, j, r] = qbT[d, a0+j] + kT[d, r]  (one broadcast add)
                in0 = qbT_sb[:, a0:a0 + CA].unsqueeze(2).broadcast_to([D, CA, R])
                in1 = kT_sb[:].unsqueeze(1).broadcast_to([D, CA, R])
                nc.vector.tensor_add(e[:], in0, in1)
                nc.scalar.activation(th[:], e[:], TANH)
                for j in range(CA):
                    nc.tensor.matmul(
                        sc_ps[:, a0 + j:a0 + j + 1],
                        th[:, j],
                        a_f16[:],
                        start=True,
                        stop=True,
                    )
            sc_sb = scs.tile([R, A], F32, tag="sc_sb")
            nc.vector.tensor_copy(sc_sb[:], sc_ps[:])
            nc.sync.dma_start(out[b], sc_sb[:])

    nc.finalize()
    return nc


def _get_nc():
    if "nc" not in _CACHE:
        _CACHE["nc"] = build_bass()
    return _CACHE["nc"]


def make_in_maps(inputs):
    receiver = np.ascontiguousarray(inputs["receiver"], dtype=np.float32)
    attendant = np.ascontiguousarray(inputs["attendant"], dtype=np.float32)
    w_q = np.ascontiguousarray(inputs["w_q"], dtype=np.float32)
    w_k = np.ascontiguousarray(inputs["w_k"], dtype=np.float32)
    bias = np.ascontiguousarray(inputs["bias"], dtype=np.float32).reshape(D, 1)
    a = np.ascontiguousarray(inputs["a"], dtype=np.float32).reshape(D, 1)
    in_maps = []
    for c in range(NCORES):
        in_maps.append(
            {
                "receiver": np.ascontiguousarray(receiver[c * BC:(c + 1) * BC]),
                "attendant": np.ascontiguousarray(attendant[c * BC:(c + 1) * BC]),
                "w_q": w_q,
                "w_k": w_k,
                "bias": bias,
                "a": a,
            }
        )
    return in_maps


def run(inputs, **kwargs):
    nc = _get_nc()
    in_maps = make_in_maps(inputs)
    res = run_bass_kernel_spmd(nc, in_maps, list(range(NCORES)), **kwargs)
    out = np.concatenate([res.results[c]["out"] for c in range(NCORES)], axis=0)
    return out, res


def kernel(**inputs) -> np.ndarray:
    out, _ = run(inputs)
    return out
